# revision 40
# baseline (speedup 1.0000x reference)
"""CS-TreeLSTM (BRANCH=16, DEPTH=4, IN=HID=512) on 8 Trainium2 NeuronCores.

Strategy (data-parallel over subtrees, per the sharding hint):
  - Each core owns 8192 leaves, 512 level-3, 32 level-2, 2 level-1 nodes.
  - Activations live transposed on-chip: [hid/in on partitions, nodes on free].
  - Gate matmuls run as f32r (fp32-reduced, ~tf32) at bf16 PE speed.
  - Sibling sums (h_sum, sum_k f_k*C_k) are grouped free-dim reduces on DVE.
  - The parent-x term of the forget gate is folded into the PE accumulation via
    an indicator-matrix matmul (fx in natural layout as stationary, S moving).
  - Level 1 runs with natural-layout gates (nodes on partitions) to keep N=512.
  - Level 0 (one root; children span all cores) is combined on the host from
    per-core H1/C1 outputs (the only cross-core communication, 8x8KB).

Built on bacc.Bacc so multi-semaphore waits are legalized into event
semaphores automatically (TRN2 allows one sync wait per instruction).
"""

import sys

sys.path.insert(0, "/opt/trn_rl_repo")

import numpy as np

BRANCH = 16
DEPTH = 4
IN = 512
HID = 512
NC_N = 8
SIZES = [BRANCH**d for d in range(DEPTH + 1)]  # [1,16,256,4096,65536]
OFFS = [0, 1, 17, 273, 4369, 69905]
XT_COLS = 8192 + 512 + 32 + 2  # 8738
C3_OFF = 8192
C2_OFF = 8192 + 512
N_CHUNK = 16

_CACHE = {}


def _build_nc(cfg=None):
    cfg = cfg or {}
    from concourse import bacc
    import concourse.mybir as mybir
    import concourse.tile as tile
    from concourse.masks import make_identity

    F32 = mybir.dt.float32
    F32R = mybir.dt.float32r
    ACTF = mybir.ActivationFunctionType
    AX = mybir.AxisListType
    OP = mybir.AluOpType

    nc = bacc.Bacc()

    xt = nc.declare_dram_parameter("xt", [IN, XT_COLS], F32, isOutput=False)
    wname = ["wix", "wih", "wox", "woh", "wux", "wuh", "wfx", "wfh"]
    wps = {n: nc.declare_dram_parameter(n, [IN, HID], F32, isOutput=False) for n in wname}
    bT = {g: nc.declare_dram_parameter("bT" + g, [128, 4], F32, isOutput=False) for g in "iouf"}
    bD = nc.declare_dram_parameter("bD", [128, 4, 8], F32, isOutput=False)
    S_p = nc.declare_dram_parameter("S", [128, 512], F32, isOutput=False)
    out_hc = nc.declare_dram_parameter("out_hc", [4, HID], F32, isOutput=True)

    def t_view(h):  # DRAM [512, n] -> [128 part, 4 ktile, n] view
        return h[:, :].rearrange("(t p) n -> p t n", p=128)

    from contextlib import ExitStack

    with tile.TileContext(nc) as tc, ExitStack() as ctx:
        consts = ctx.enter_context(tc.tile_pool(name="consts", bufs=1))
        stream = ctx.enter_context(tc.tile_pool(name="stream", bufs=cfg.get("stream", 5)))
        workA = ctx.enter_context(tc.tile_pool(name="workA", bufs=cfg.get("workA", 2)))
        workB = ctx.enter_context(tc.tile_pool(name="workB", bufs=cfg.get("workB", 2)))
        longp = ctx.enter_context(tc.tile_pool(name="longp", bufs=1))
        psum = ctx.enter_context(tc.tile_pool(name="psum", bufs=cfg.get("psum", 4), space="PSUM"))
        psum_half = cfg.get("psum_half", False)

        # ---------------- constants / weights ----------------
        # DMA order is the startup critical path: first chunk's x, then the
        # i/o/u x-part weights, then fx3 inputs, then the rest. The h-part
        # weights (wih/woh/wuh) are only needed at level 3, so they ride the
        # leaf x stream pool late instead of holding SBUF all along.
        W = {}
        bTs = {}
        stream_tiles = {}

        def load_w(n):
            W[n] = consts.tile([128, 4, HID], F32R, tag="w_" + n, name="w_" + n)
            nc.sync.dma_start(out=W[n][:, :, :], in_=t_view(wps[n]).bitcast(F32R))

        def load_chunk(c):
            t = stream.tile([128, 4, 512], F32R, tag="xt_c", name=f"xt_c{c}")
            nc.sync.dma_start(
                out=t[:, :, :], in_=t_view(xt)[:, :, c * 512 : (c + 1) * 512].bitcast(F32R)
            )
            stream_tiles[c] = t
            return t

        # chunk 0 and wix stream per k-tile, interleaved, so the first gate's
        # k-waves start as soon as each 0.5 MB pair lands
        t0 = stream.tile([128, 4, 512], F32R, tag="xt_c", name="xt_c0")
        stream_tiles[0] = t0
        W["wix"] = consts.tile([128, 4, HID], F32R, tag="w_wix", name="w_wix")
        for k in range(4):
            nc.sync.dma_start(out=t0[:, k, :], in_=t_view(xt)[:, k, 0:512].bitcast(F32R))
            nc.sync.dma_start(out=W["wix"][:, k, :], in_=t_view(wps["wix"])[:, k, :].bitcast(F32R))
        for g in "iouf":
            bTs[g] = consts.tile([128, 4], F32, tag="bT" + g, name="bT" + g)
            nc.sync.dma_start(out=bTs[g][:, :], in_=bT[g][:, :])
        load_w("wox")
        load_w("wux")
        xt3 = stream.tile([128, 4, 512], F32R, tag="xt_c", name="xt3")
        nc.sync.dma_start(out=xt3[:, :, :], in_=t_view(xt)[:, :, C3_OFF : C3_OFF + 512].bitcast(F32R))
        load_w("wfx")
        load_chunk(1)
        load_w("wfh")
        S_sb = consts.tile([128, 512], F32R, tag="S")
        nc.sync.dma_start(out=S_sb[:, :], in_=S_p[:, :].bitcast(F32R))
        bD_sb = consts.tile([128, 4, 8], F32, tag="bD")
        nc.sync.dma_start(out=bD_sb[:, :, :], in_=bD[:, :, :])
        ident = consts.tile([128, 128], F32, tag="ident")
        make_identity(nc, ident[:, :])

        # resident x tail (level-2/1 x)
        xt_tail = consts.tile([128, 4, 34], F32R, tag="xt_tail")
        nc.sync.dma_start(out=xt_tail[:, :, :], in_=t_view(xt)[:, :, C2_OFF : C2_OFF + 34].bitcast(F32R))

        # persistent accumulators
        hsum3T = longp.tile([128, 4, 512], F32R, tag="hsum3T")
        fcsum3T = longp.tile([128, 4, 512], F32, tag="fcsum3T")
        fx3_nat = longp.tile([128, 4, 512], F32R, tag="fx3_nat")

        def gate_T(g, rhs_x, rhs_h, n, aug=None):
            """Transposed-layout gate accumulation into a fresh psum tile.
            g in {i,o,u}: pre[:,m,:n] = sum_k WgxT[k,m].T @ rhs_x[k]
                          (+ sum_k WghT[k,m].T @ rhs_h[k]) (+ aug)
            g == "fh":    pre[:,m,:n] = sum_k WfhT[k,m].T @ rhs_x[k] (+ aug)"""
            pa = psum.tile([128, 2, 512], F32, tag="ps", name="ps_a")
            pb2 = psum.tile([128, 2, 512], F32, tag="ps", name="ps_b")
            ps = (pa, pb2)

            def slot(m):
                return ps[m // 2][:, m % 2, :n]

            if rhs_h is not None:
                # all x-part matmuls first (independent), then the h-part wave:
                # the first h matmul may wait on hsum, covered by 16 x matmuls
                for m in range(4):
                    ms = slice(m * 128, (m + 1) * 128)
                    for k in range(4):
                        nc.tensor.matmul(
                            slot(m), W["w" + g + "x"][:, k, ms], rhs_x[:, k, :],
                            start=(k == 0), stop=False,
                        )
                for m in range(4):
                    ms = slice(m * 128, (m + 1) * 128)
                    for k in range(4):
                        nc.tensor.matmul(
                            slot(m), W["w" + g + "h"][:, k, ms], rhs_h[:, k, :],
                            start=False, stop=(k == 3),
                        )
                return ps
            for m in range(4):
                ms = slice(m * 128, (m + 1) * 128)
                if g == "fh":
                    seq = [(W["wfh"], rhs_x, k) for k in range(4)]
                else:
                    seq = [(W["w" + g + "x"], rhs_x, k) for k in range(4)]
                naug = 1 if aug is not None else 0
                for idx, (wt, rhs, k) in enumerate(seq):
                    nc.tensor.matmul(
                        slot(m), wt[:, k, ms], rhs[:, k, :],
                        start=(idx == 0), stop=(naug == 0 and idx == len(seq) - 1),
                    )
                if naug:
                    la, ra, base = aug(ms)
                    nc.tensor.matmul(
                        slot(m), la, ra, start=False, stop=True,
                        tile_position=(base, 0),
                    )
            return ps

        def evac(ps, n, act, bias_g, out_sb):
            for m in range(4):
                b = 0.0 if bias_g is None else bTs[bias_g][:, m : m + 1]
                nc.scalar.activation(out_sb[:, m, :n], ps[m // 2][:, m % 2, :n], act, bias=b)

        LOWP = "f32r rounding for downstream matmul"

        def fx3_compute():
            # fx3_nat = x3 @ WfxT (natural layout)
            psx = (psum.tile([128, 2, 512], F32, tag="ps", name="ps_a"),
                   psum.tile([128, 2, 512], F32, tag="ps", name="ps_b"))
            for pb in range(4):
                for k in range(4):
                    nc.tensor.matmul(
                        psx[pb // 2][:, pb % 2, :], xt3[:, k, pb * 128 : (pb + 1) * 128],
                        W["wfx"][:, k, :], start=(k == 0), stop=(k == 3),
                    )
            for pb in range(4):
                nc.scalar.activation(fx3_nat[:, pb, :], psx[pb // 2][:, pb % 2, :], ACTF.Copy)

        # ---------------- leaf phase ----------------
        # The f-gate matmuls for chunk c need C(c) (a DVE product of ACT
        # outputs); running them one chunk behind keeps PE from stalling on
        # the ACT/DVE tail of the current chunk.
        def leaf_fpath(c, C_prev):
            b, pt = 32 * (c % 4), c // 4

            def aug3(ms, b=b, pt=pt):
                return fx3_nat[b : b + 32, pt, ms], S_sb[b : b + 32, :], b

            ps_f = gate_T("fh", C_prev, None, 512, aug=aug3)
            f_sb = workB.tile([128, 4, 512], F32, tag="Ug")
            evac(ps_f, 512, ACTF.Sigmoid, "f", f_sb)
            fC_sb = workB.tile([128, 4, 512], F32, tag="H")
            for m in range(4):
                nc.vector.tensor_mul(fC_sb[:, m, :], f_sb[:, m, :], C_prev[:, m, :].bitcast(F32))
            for m in range(4):
                nc.vector.tensor_reduce(
                    fcsum3T[:, m, 32 * c : 32 * c + 32],
                    fC_sb[:, m, :].rearrange("p (g w) -> p g w", w=16),
                    axis=AX.X, op=OP.add,
                )

        def leaf_hpath(c, C_prev, o_prev):
            tC_sb = workA.tile([128, 4, 512], F32, tag="A")
            H_sb = workB.tile([128, 4, 512], F32, tag="H")
            for m in range(4):
                nc.scalar.activation(tC_sb[:, m, :], C_prev[:, m, :].bitcast(F32), ACTF.Tanh)
            for m in range(4):
                nc.vector.tensor_mul(H_sb[:, m, :], o_prev[:, m, :], tC_sb[:, m, :])
            with nc.allow_low_precision(LOWP):
                for m in range(4):
                    nc.vector.tensor_reduce(
                        hsum3T[:, m, 32 * c : 32 * c + 32],
                        H_sb[:, m, :].rearrange("p (g w) -> p g w", w=16),
                        axis=AX.X, op=OP.add,
                    )

        pipe = None  # (chunk index, C_sb, o_sb)
        for c in range(N_CHUNK):
            xt_c = stream_tiles[c] if c in stream_tiles else load_chunk(c)

            if c == 0:
                # k-outer so each arriving (x, wix) k-tile pair is consumed
                ps_i = (psum.tile([128, 2, 512], F32, tag="ps", name="ps_a"),
                        psum.tile([128, 2, 512], F32, tag="ps", name="ps_b"))
                for k in range(4):
                    for m in range(4):
                        nc.tensor.matmul(
                            ps_i[m // 2][:, m % 2, :], W["wix"][:, k, m * 128 : (m + 1) * 128],
                            xt_c[:, k, :], start=(k == 0), stop=(k == 3),
                        )
            else:
                ps_i = gate_T("i", xt_c, None, 512)
            i_sb = workA.tile([128, 4, 512], F32, tag="A")
            evac(ps_i, 512, ACTF.Sigmoid, "i", i_sb)

            ps_o = gate_T("o", xt_c, None, 512)
            o_sb = workB.tile([128, 4, 512], F32, tag="B")
            evac(ps_o, 512, ACTF.Sigmoid, "o", o_sb)

            ps_u = gate_T("u", xt_c, None, 512)
            u_sb = workB.tile([128, 4, 512], F32, tag="Ug")
            evac(ps_u, 512, ACTF.Tanh, "u", u_sb)

            C_sb = workA.tile([128, 4, 512], F32R, tag="C")
            for m in range(4):
                nc.vector.tensor_mul(C_sb[:, m, :], i_sb[:, m, :], u_sb[:, m, :])

            if c == 0:
                fx3_compute()
            if pipe is not None and not cfg.get("no_fpath"):
                leaf_fpath(pipe[0], pipe[1])

            if not cfg.get("no_hpath"):
                leaf_hpath(c, C_sb, o_sb)
            pipe = (c, C_sb, o_sb)

        leaf_fpath(pipe[0], pipe[1])

        # late-loaded h-part weights (ride the stream pool slots)
        for n in ("wih", "woh", "wuh"):
            W[n] = stream.tile([128, 4, HID], F32R, tag="xt_c", name="w_" + n)
            nc.sync.dma_start(out=W[n][:, :, :], in_=t_view(wps[n]).bitcast(F32R))

        # ---------------- level 3 (512 nodes, transposed) ----------------
        ps3 = gate_T("i", xt3, hsum3T, 512)
        i3 = workA.tile([128, 4, 512], F32, tag="A")
        evac(ps3, 512, ACTF.Sigmoid, "i", i3)
        ps3 = gate_T("o", xt3, hsum3T, 512)
        o3 = workB.tile([128, 4, 512], F32, tag="B")
        evac(ps3, 512, ACTF.Sigmoid, "o", o3)
        ps3 = gate_T("u", xt3, hsum3T, 512)
        u3 = workB.tile([128, 4, 512], F32, tag="Ug")
        evac(ps3, 512, ACTF.Tanh, "u", u3)

        # fx2_nat [32,512] and fx1_nat [2,512] (independent of the leaf/L3
        # dataflow; emitted here so PE stays busy while C3 is produced)
        ps = psum.tile([128, 2, 512], F32, tag="ps", name="ps_a")
        for k in range(4):
            nc.tensor.matmul(
                ps[0:32, 0, :], xt_tail[:, k, 0:32], W["wfx"][:, k, :], start=(k == 0), stop=(k == 3)
            )
        for k in range(4):
            nc.tensor.matmul(
                ps[0:2, 1, :], xt_tail[:, k, 32:34], W["wfx"][:, k, :], start=(k == 0), stop=(k == 3)
            )
        fx2_nat = longp.tile([128, 512], F32R, tag="fx2_nat")
        nc.scalar.activation(fx2_nat[0:32, :], ps[0:32, 0, :], ACTF.Copy)
        fx1_nat = longp.tile([128, 512], F32R, tag="fx1_nat")
        nc.scalar.activation(fx1_nat[0:2, :], ps[0:2, 1, :], ACTF.Copy)

        C3 = workA.tile([128, 4, 512], F32R, tag="C")
        iu3 = workB.tile([128, 4, 512], F32, tag="H")
        for m in range(4):
            nc.vector.tensor_mul(iu3[:, m, :], i3[:, m, :], u3[:, m, :])
            with nc.allow_low_precision(LOWP):
                nc.vector.tensor_add(C3[:, m, :], iu3[:, m, :], fcsum3T[:, m, :])
        tC3 = workA.tile([128, 4, 512], F32, tag="A")
        H3 = workB.tile([128, 4, 512], F32, tag="H")
        for m in range(4):
            nc.scalar.activation(tC3[:, m, :], C3[:, m, :].bitcast(F32), ACTF.Tanh)
        for m in range(4):
            nc.vector.tensor_mul(H3[:, m, :], o3[:, m, :], tC3[:, m, :])

        # ---------------- f-path to level 2 ----------------
        # fx2/fx1 were computed right after the level-3 gates (independent
        # work that fills the PE gap while C3 is produced).

        def aug2(ms):
            return fx2_nat[0:32, ms], S_sb[0:32, :], 0

        ps_f2 = gate_T("fh", C3, None, 512, aug=aug2)
        f2 = workB.tile([128, 4, 512], F32, tag="Ug")
        evac(ps_f2, 512, ACTF.Sigmoid, "f", f2)
        fC2 = workB.tile([128, 4, 512], F32, tag="B")
        for m in range(4):
            nc.vector.tensor_mul(fC2[:, m, :], f2[:, m, :], C3[:, m, :].bitcast(F32))

        hsum2T = longp.tile([128, 4, 32], F32R, tag="hsum2T")
        fcsum2T = longp.tile([128, 4, 32], F32, tag="fcsum2T")
        with nc.allow_low_precision(LOWP):
            nc.vector.tensor_reduce(
                hsum2T[:, :, :],
                H3[:, :, :].rearrange("p t (g w) -> p t g w", w=16),
                axis=AX.X, op=OP.add,
            )
        nc.vector.tensor_reduce(
            fcsum2T[:, :, :],
            fC2[:, :, :].rearrange("p t (g w) -> p t g w", w=16),
            axis=AX.X, op=OP.add,
        )

        # ---------------- level 2 (32 nodes, transposed) ----------------
        x2v = xt_tail[:, :, 0:32]
        ps2 = gate_T("i", x2v, hsum2T, 32)
        i2 = longp.tile([128, 4, 32], F32, tag="s_i")
        evac(ps2, 32, ACTF.Sigmoid, "i", i2)
        ps2 = gate_T("o", x2v, hsum2T, 32)
        o2 = longp.tile([128, 4, 32], F32, tag="s_o")
        evac(ps2, 32, ACTF.Sigmoid, "o", o2)
        ps2 = gate_T("u", x2v, hsum2T, 32)
        u2 = longp.tile([128, 4, 32], F32, tag="s_u")
        evac(ps2, 32, ACTF.Tanh, "u", u2)

        # level-1 gate x-parts hoisted here (independent of the level-2
        # ladder) into held-open psum groups; h-parts + bias land later
        l1T1 = psum.tile([128, 2, 512], F32, tag="ps", name="ps_a")
        l1T2 = psum.tile([128, 2, 512], F32, tag="ps", name="ps_b")
        l1slot = {"i": l1T1[0:2, 0, :], "o": l1T1[0:2, 1, :], "u": l1T2[0:2, 0, :]}
        for g in "iou":
            for k in range(4):
                nc.tensor.matmul(
                    l1slot[g], xt_tail[:, k, 32:34], W["w" + g + "x"][:, k, :],
                    start=(k == 0), stop=False,
                )

        C2 = longp.tile([128, 4, 32], F32R, tag="C2")
        iu2 = longp.tile([128, 4, 32], F32, tag="s_t")
        nc.vector.tensor_mul(iu2[:, :, :], i2[:, :, :], u2[:, :, :])
        with nc.allow_low_precision(LOWP):
            nc.vector.tensor_add(C2[:, :, :], iu2[:, :, :], fcsum2T[:, :, :])
        tC2 = longp.tile([128, 4, 32], F32, tag="s_t2")
        nc.scalar.activation(tC2[:, :, :], C2[:, :, :].bitcast(F32), ACTF.Tanh)
        H2 = longp.tile([128, 4, 32], F32, tag="s_h")
        nc.vector.tensor_mul(H2[:, :, :], o2[:, :, :], tC2[:, :, :])

        # ---------------- f-path to level 1 ----------------

        def aug1(ms):
            return fx1_nat[0:2, ms], S_sb[0:2, 0:32], 0

        ps_f1 = gate_T("fh", C2, None, 32, aug=aug1)
        f1 = longp.tile([128, 4, 32], F32, tag="s_f1")
        evac(ps_f1, 32, ACTF.Sigmoid, "f", f1)
        fC1 = longp.tile([128, 4, 32], F32, tag="s_fc1")
        nc.vector.tensor_mul(fC1[:, :, :], f1[:, :, :], C2[:, :, :].bitcast(F32))

        hsum1T = longp.tile([128, 4, 2], F32R, tag="hsum1T")
        fcsum1T = longp.tile([128, 4, 2], F32, tag="fcsum1T")
        with nc.allow_low_precision(LOWP):
            nc.vector.tensor_reduce(
                hsum1T[:, :, :],
                H2[:, :, :].rearrange("p t (g w) -> p t g w", w=16),
                axis=AX.X, op=OP.add,
            )
        nc.vector.tensor_reduce(
            fcsum1T[:, :, :],
            fC1[:, :, :].rearrange("p t (g w) -> p t g w", w=16),
            axis=AX.X, op=OP.add,
        )

        # ---------------- level 1 (2 nodes, natural-layout gates) ----------------
        # [2,512] scratch tensors share two tiles via free-dim offsets (DVE
        # two-input ops require equal base partitions, so all sit at rows 0:2).
        buf_a = longp.tile([128, 1024], F32, tag="buf_a")
        buf_b = longp.tile([128, 1024], F32, tag="buf_b")
        g1 = {}
        for gi, (g, act) in enumerate(
            (("i", ACTF.Sigmoid), ("o", ACTF.Sigmoid), ("u", ACTF.Tanh))
        ):
            for k in range(4):
                nc.tensor.matmul(
                    l1slot[g], hsum1T[:, k, :], W["w" + g + "h"][:, k, :],
                    start=False, stop=False,
                )
            # bias: transpose a duplicated bias column into both node rows
            bidx = "iouf".index(g)
            for m in range(4):
                nc.tensor.matmul(
                    l1slot[g][:, m * 128 : (m + 1) * 128],
                    bD_sb[:, m, 2 * bidx : 2 * bidx + 2], ident[:, :],
                    is_transpose=True, start=False, stop=(m == 3),
                )
            g1[g] = (buf_a[0:2, 0:512], buf_a[0:2, 512:1024], buf_b[0:2, 512:1024])[gi]
            nc.scalar.activation(g1[g], l1slot[g], act)

        # fcsum1 to natural layout via PE transpose
        ps_t = psum.tile([128, 2, 512], F32, tag="ps", name="ps_a")
        for t in range(4):
            nc.tensor.transpose(ps_t[0:2, 0, t * 128 : (t + 1) * 128], fcsum1T[:, t, :], ident[:, :])

        iu1 = buf_b[0:2, 0:512]
        nc.vector.tensor_mul(iu1, g1["i"], g1["u"])
        C1 = buf_b[0:2, 512:1024]
        nc.vector.tensor_add(C1, iu1, ps_t[0:2, 0, :])
        tC1 = buf_a[0:2, 0:512]
        nc.scalar.activation(tC1, C1, ACTF.Tanh)
        H1 = buf_b[0:2, 0:512]
        nc.vector.tensor_mul(H1, g1["o"], tC1)

        nc.sync.dma_start(out=out_hc[0:2, :], in_=H1)
        nc.sync.dma_start(out=out_hc[2:4, :], in_=C1)

    nc.finalize()
    return nc


def _np_sigmoid(v):
    return 1.0 / (1.0 + np.exp(-v))


def _host_prep(x, wi_w, wo_w, wu_w, wf_w, wi_b, wo_b, wu_b, wf_b):
    xt_full = np.ascontiguousarray(x.T)  # [512, 69905]

    def wT(w, part):
        return np.ascontiguousarray(w[:, :512].T if part == "x" else w[:, 512:].T)

    common = {
        "wix": wT(wi_w, "x"), "wih": wT(wi_w, "h"),
        "wox": wT(wo_w, "x"), "woh": wT(wo_w, "h"),
        "wux": wT(wu_w, "x"), "wuh": wT(wu_w, "h"),
        "wfx": wT(wf_w, "x"), "wfh": wT(wf_w, "h"),
        "bTi": np.ascontiguousarray(np.asarray(wi_b).reshape(4, 128).T),
        "bTo": np.ascontiguousarray(np.asarray(wo_b).reshape(4, 128).T),
        "bTu": np.ascontiguousarray(np.asarray(wu_b).reshape(4, 128).T),
        "bTf": np.ascontiguousarray(np.asarray(wf_b).reshape(4, 128).T),
        "bD": np.ascontiguousarray(
            np.stack([np.asarray(b).reshape(4, 128).T for b in (wi_b, wo_b, wu_b, wf_b)], axis=2)
            .repeat(2, axis=2).reshape(128, 4, 8)
        ),
        "S": (np.arange(512)[None, :] // 16 == (np.arange(128) % 32)[:, None]).astype(np.float32),
    }
    in_maps = []
    for c in range(NC_N):
        xt_c = np.concatenate(
            [
                xt_full[:, OFFS[4] + 8192 * c : OFFS[4] + 8192 * (c + 1)],
                xt_full[:, OFFS[3] + 512 * c : OFFS[3] + 512 * (c + 1)],
                xt_full[:, OFFS[2] + 32 * c : OFFS[2] + 32 * (c + 1)],
                xt_full[:, OFFS[1] + 2 * c : OFFS[1] + 2 * (c + 1)],
            ],
            axis=1,
        )
        in_maps.append({"xt": np.ascontiguousarray(xt_c), **common})
    return in_maps


def _host_finish(x, H1_all, C1_all, wi_w, wi_b, wf_w, wf_b, wo_w, wo_b, wu_w, wu_b):
    """Level 0 (root): its 16 children are the level-1 nodes across cores."""
    f8 = np.float64
    x0 = np.asarray(x[0], f8)
    H1 = np.asarray(H1_all, f8)
    C1 = np.asarray(C1_all, f8)
    hsum0 = H1.sum(0)
    f0 = _np_sigmoid(
        x0 @ np.asarray(wf_w, f8)[:, :512].T + C1 @ np.asarray(wf_w, f8)[:, 512:].T + np.asarray(wf_b, f8)
    )
    fcsum0 = (f0 * C1).sum(0)
    xh0 = np.concatenate([x0, hsum0])
    i0 = _np_sigmoid(xh0 @ np.asarray(wi_w, f8).T + np.asarray(wi_b, f8))
    o0 = _np_sigmoid(xh0 @ np.asarray(wo_w, f8).T + np.asarray(wo_b, f8))
    u0 = np.tanh(xh0 @ np.asarray(wu_w, f8).T + np.asarray(wu_b, f8))
    C0 = i0 * u0 + fcsum0
    H0 = o0 * np.tanh(C0)
    return H0.astype(np.float32), C0.astype(np.float32)


def _run(in_maps, trace=False):
    from concourse.bass_utils import run_bass_kernel_spmd

    if "nc" not in _CACHE:
        _CACHE["nc"] = _build_nc()
    return run_bass_kernel_spmd(_CACHE["nc"], in_maps, list(range(NC_N)), trace=trace)


def kernel(x, wi_w, wi_b, wf_w, wf_b, wo_w, wo_b, wu_w, wu_b, _trace=False):
    x = np.asarray(x, np.float32)
    in_maps = _host_prep(x, wi_w, wo_w, wu_w, wf_w, wi_b, wo_b, wu_b, wf_b)
    res = _run(in_maps, trace=_trace)
    _CACHE["last_results"] = res
    H1_all = np.concatenate([res.results[c]["out_hc"][0:2] for c in range(NC_N)])
    C1_all = np.concatenate([res.results[c]["out_hc"][2:4] for c in range(NC_N)])
    H0, C0 = _host_finish(x, H1_all, C1_all, wi_w, wi_b, wf_w, wf_b, wo_w, wo_b, wu_w, wu_b)
    return H0, C0
